# revision 2
# baseline (speedup 1.0000x reference)
"""ContextualConv2d Trainium2 kernel.

Problem: grouped 3x3 conv (N=32, 128ci -> 256co, groups=4, 56x56, pad 1)
plus per-(batch,channel) context bias: out = conv(x, w) + (c @ cwT)[n,co]
+ bias[co].

Sharding (8 cores): core = (group-pair gp in {0,1}) x (batch quarter q in
{0..3}). Each core computes 8 images x 128 out-channels (2 groups of 64).

Packing: the contraction of each group's conv is (ci=32) x (kh=3) x
(kw=3) = 288. We put (kh, ci) on 96 SBUF partitions -- three row-shifted
copies of the image, shift baked in at DMA-load time -- and handle kw
with the moving-operand column offset. That gives a 100%-dense [96, 64]
lhsT, so each (img, 8x56 tile) needs 6 matmul passes of N=448 (3 kw x 2
groups) instead of the 9 half-dense block-diagonal passes: 1.5x less PE
time. The 3x HBM read amplification on x is paid for by moving x and y
to bf16 (PSUM still accumulates fp32; ctx/bias stay fp32).

Epilogue fuses the context/bias add (per-partition scalar) with the
PSUM->SBUF copy, alternating DVE/ACT engines; output stored bf16 and
upcast on host.
"""

import numpy as np
import ml_dtypes

from concourse import bass, mybir, tile
from concourse.vector_clock import ScopedClock
from concourse.bass_utils import run_bass_kernel_spmd

BF16 = ml_dtypes.bfloat16

N, CIN, H, W = 32, 128, 56, 56
COUT, KH, KW = 256, 3, 3
GROUPS = 4
CDIM = 64
HP, WP = H + 2, W + 2
ROWS = 8              # output rows per n-tile
NT = H // ROWS        # 7 n-tiles per image
NFREE = ROWS * W      # 448 <= 512 fp32 PSUM bank limit
N_CORES = 8
IMGS = N // 4         # 8 images per core
CI = CIN // 2         # 64 input channels per core (2 groups)
CO = COUT // 2        # 128 output channels per core (2 groups)
KP = KH * 32          # 96 contraction partitions per group (kh x ci)


class _TC(tile.TileContext):
    """This container's walrus accepts only one sem wait on a Drain
    (CTRL) instruction; TileContext's tail drain aggregates one wait per
    outstanding semaphore. Split them across sequential drains."""

    def _drain_and_barrier(self, tick_clock, wait_clock):
        drain_inst = self.nc.sync.drain()
        wait_clock.add_sem_waits(
            drain_inst.ins, ScopedClock({None: tick_clock.global_clock})
        )
        si = drain_inst.ins.sync_info
        if si is not None and len(si.on_wait) > 1:
            waits = list(si.on_wait)
            si.on_wait.clear()
            si.on_wait.append(waits[0])
            for w in waits[1:]:
                d2 = self.nc.sync.drain()
                d2.ins.sync_info = mybir.SyncInfo(on_wait=[w], on_update=[])
        self.nc.all_engine_barrier()
        assert self.sems is not None
        popped = self.nc._tile_sem_poison_stack.pop()
        assert popped is self._sem_poison
        self.nc.clear_and_free_semaphores(list(self.sems.allocated().values()))
        self.nc.all_engine_barrier()


_ws_ctr = [0]


def _split_waits(nc):
    """Walrus here caps sem waits at one per instruction; hoist extras
    onto injected same-engine NoOps placed just before the owner."""
    for fn in nc.m.functions:
        for blk in fn.blocks:
            insts = blk.instructions
            out = []
            changed = False
            for inst in insts:
                si = getattr(inst, "sync_info", None)
                if si is not None and si.on_wait and len(si.on_wait) > 1:
                    waits = list(si.on_wait)
                    for w in waits[:-1]:
                        _ws_ctr[0] += 1
                        out.append(
                            mybir.InstNoOp(
                                name=f"WSNOP-{_ws_ctr[0]}",
                                engine=inst.engine,
                                ins=[],
                                outs=[],
                                sync_info=mybir.SyncInfo(on_wait=[w], on_update=[]),
                                debug=inst.debug,
                            )
                        )
                        changed = True
                    si.on_wait.clear()
                    si.on_wait.append(waits[-1])
                out.append(inst)
            if changed:
                insts.clear()
                insts.extend(out)
    return nc


def build_program(loop_n: int = 0):
    """loop_n > 0 builds a benchmark variant: the conv body repeats
    loop_n times inside a hardware For_i so device time dominates the
    (RPC/transfer-heavy) wall clock. loop_n=0 is the production kernel."""
    f32 = mybir.dt.float32
    f32r = mybir.dt.float32r
    bf16 = mybir.dt.bfloat16
    nc = bass.Bass("TRN2", target_bir_lowering=False, debug=False)
    xs = nc.declare_dram_parameter("xs", [IMGS, CI, HP, WP], bf16, isOutput=False)
    wk = nc.declare_dram_parameter("wk", [KP, 6, 64], bf16, isOutput=False)
    cwb = nc.declare_dram_parameter("cwb", [CDIM + 1, CO], f32r, isOutput=False)
    cb = nc.declare_dram_parameter("cb", [CDIM + 1, IMGS], f32r, isOutput=False)
    y = nc.declare_dram_parameter("y", [IMGS, CO, H * W], bf16, isOutput=True)

    with _TC(nc) as tc:
        with (
            tc.tile_pool(name="wp", bufs=1) as wpool,
            tc.tile_pool(name="xp", bufs=4) as xpool,
            tc.tile_pool(name="op", bufs=4) as opool,
            tc.tile_pool(name="psa", bufs=3, space="PSUM") as psapool,
            tc.tile_pool(name="psb", bufs=3, space="PSUM") as psbpool,
            tc.tile_pool(name="psc", bufs=1, space="PSUM") as pscpool,
        ):
            wt = wpool.tile([KP, 6, 64], bf16)
            nc.sync.dma_start(wt[:], wk[:])
            cwbt = wpool.tile([CDIM + 1, CO], f32r)
            nc.sync.dma_start(cwbt[:], cwb[:])
            cbt = wpool.tile([CDIM + 1, IMGS], f32r)
            nc.sync.dma_start(cbt[:], cb[:])

            # bctx[co_loc, g*IMGS+n] = sum_d c_weight[co,d] c[n,d] + bias[co]
            psc = pscpool.tile([64, 2 * IMGS], f32)
            nc.tensor.matmul(psc[:, 0:IMGS], cwbt[:, 0:64], cbt[:], start=True, stop=True)
            nc.tensor.matmul(psc[:, IMGS:], cwbt[:, 64:CO], cbt[:], start=True, stop=True)
            bctx = wpool.tile([64, 2 * IMGS], f32)
            nc.vector.tensor_copy(bctx[:], psc[:, :])

            def conv_body():
                for i in range(IMGS):
                    # partition block kh holds the image shifted up kh rows:
                    # xg[32*kh+ci, r, w] = xpad[ci, r+kh, w]
                    xga = xpool.tile([KP, HP, WP], bf16, name=f"xga{i}", tag="xg")
                    xgb = xpool.tile([KP, HP, WP], bf16, name=f"xgb{i}", tag="xg")
                    for kh in range(KH):
                        nc.sync.dma_start(
                            xga[32 * kh : 32 * kh + 32, 0 : HP - kh, :],
                            xs[i, 0:32, kh:HP, :],
                        )
                        nc.sync.dma_start(
                            xgb[32 * kh : 32 * kh + 32, 0 : HP - kh, :],
                            xs[i, 32:CI, kh:HP, :],
                        )
                    otA = opool.tile([64, H * W], bf16, name=f"otA{i}", tag="ot")
                    otB = opool.tile([64, H * W], bf16, name=f"otB{i}", tag="ot")
                    for t in range(NT):
                        psA = psapool.tile([64, NFREE], f32, name=f"psA{i}_{t}", tag="psa")
                        psB = psbpool.tile([64, NFREE], f32, name=f"psB{i}_{t}", tag="psb")
                        h0 = t * ROWS
                        for kw in range(KW):
                            nc.tensor.matmul(
                                psA[:, :],
                                wt[:, kw, :],
                                xga[0:KP, h0 : h0 + ROWS, kw : kw + W],
                                start=(kw == 0),
                                stop=(kw == 2),
                            )
                        for kw in range(KW):
                            nc.tensor.matmul(
                                psB[:, :],
                                wt[:, 3 + kw, :],
                                xgb[0:KP, h0 : h0 + ROWS, kw : kw + W],
                                start=(kw == 0),
                                stop=(kw == 2),
                            )
                        oA = otA[:, t * NFREE : (t + 1) * NFREE]
                        oB = otB[:, t * NFREE : (t + 1) * NFREE]
                        if t % 2 == 0:
                            nc.vector.tensor_scalar_add(oA, psA[:, :], bctx[:, i : i + 1])
                            nc.scalar.activation(
                                oB, psB[:, :], mybir.ActivationFunctionType.Identity,
                                bias=bctx[:, IMGS + i : IMGS + i + 1],
                            )
                        else:
                            nc.scalar.activation(
                                oA, psA[:, :], mybir.ActivationFunctionType.Identity,
                                bias=bctx[:, i : i + 1],
                            )
                            nc.vector.tensor_scalar_add(
                                oB, psB[:, :], bctx[:, IMGS + i : IMGS + i + 1]
                            )
                    nc.sync.dma_start(y[i, 0:64, :], otA[:])
                    nc.sync.dma_start(y[i, 64:CO, :], otB[:])

            if loop_n > 0:
                with tc.For_i(0, loop_n, 1, hint_engines=(mybir.EngineType.PE,)):
                    conv_body()
            else:
                conv_body()
    _split_waits(nc)
    return nc


_prog_cache = {}


def _get_program():
    if "nc" not in _prog_cache:
        _prog_cache["nc"] = build_program()
    return _prog_cache["nc"]


def _shard_inputs(x, c, weight, bias, c_weight):
    """Build the per-core input dicts (pure layout prep, no math)."""
    xpad = np.zeros((N, CIN, HP, WP), np.float32)
    xpad[:, :, 1 : H + 1, 1 : W + 1] = x
    xpad16 = xpad.astype(BF16)

    # lhsT per (group-pair, g, kw): wk[kh*32+ci, g*3+kw, co] =
    # weight[128*gp + 64*g + co, ci, kh, kw]
    wks = []
    cwbs = []
    for gp in range(2):
        wsl = weight[CO * gp : CO * gp + CO]  # [128, 32, 3, 3]
        wkv = np.empty((KP, 6, 64), np.float32)
        for g in range(2):
            cosl = wsl[64 * g : 64 * g + 64]  # [64co, 32ci, 3, 3]
            for kh in range(KH):
                for kw in range(KW):
                    wkv[kh * 32 : kh * 32 + 32, g * 3 + kw, :] = cosl[:, :, kh, kw].T
        wks.append(wkv.astype(BF16))

        cwbv = np.empty((CDIM + 1, CO), np.float32)
        cwbv[:CDIM] = c_weight[CO * gp : CO * gp + CO].T
        cwbv[CDIM] = bias[CO * gp : CO * gp + CO]
        cwbs.append(cwbv)

    in_maps = []
    for core in range(N_CORES):
        gp, q = divmod(core, 4)
        cbv = np.empty((CDIM + 1, IMGS), np.float32)
        cbv[:CDIM] = c[IMGS * q : IMGS * q + IMGS].T
        cbv[CDIM] = 1.0
        in_maps.append(
            {
                "xs": np.ascontiguousarray(
                    xpad16[IMGS * q : IMGS * q + IMGS, CI * gp : CI * gp + CI]
                ),
                "wk": wks[gp],
                "cwb": cwbs[gp],
                "cb": cbv,
            }
        )
    return in_maps


def kernel(x, c, weight, bias, c_weight):
    x = np.asarray(x, np.float32)
    c = np.asarray(c, np.float32)
    weight = np.asarray(weight, np.float32)
    bias = np.asarray(bias, np.float32)
    c_weight = np.asarray(c_weight, np.float32)

    nc = _get_program()
    in_maps = _shard_inputs(x, c, weight, bias, c_weight)
    res = run_bass_kernel_spmd(nc, in_maps, list(range(N_CORES)), trace=False)

    out = np.empty((N, COUT, H, W), np.float32)
    for core in range(N_CORES):
        gp, q = divmod(core, 4)
        yv = np.asarray(res.results[core]["y"]).astype(np.float32)
        out[IMGS * q : IMGS * q + IMGS, CO * gp : CO * gp + CO] = yv.reshape(
            IMGS, CO, H, W
        )
    return out
